# revision 28
# baseline (speedup 1.0000x reference)
"""Localized (block-diagonal windowed) self-attention + residual + LayerNorm
on 8 Trainium2 NeuronCores.

Problem (hardcoded): x [B=4, S=4096, D=1024], H=16 heads, K=64 head dim,
num_window=8 -> window length Sw=512. Per (batch, window) block:
    q/k/v = xw @ W* + b*          [512, 16, 64]
    scores = q k^T / 8 per head   [512, 512]
    attn = softmax(scores)
    ctx = attn @ v
    attn_out = ctx @ Wo + bo
    out = LayerNorm(x + attn_out) * gamma + beta   (eps=1e-3)

Sharding: pure data parallelism over the 32 (batch, window) blocks, 4 per
core; weights replicated. No collectives.

Device strategy (v2): fp8-e4m3 DoubleRow perf mode (0.5 cycles/row)
everywhere the ~2e-2 error budget allows; bf16 where it does not:
  - q/k projections: bf16 (fp8 Wq/Wk quantization perturbs softmax
    weights enough to fail the tolerance on peaked-attention tokens);
    q/k outputs stored fp8 in the scores pair layout.
  - v/o projections: fp8 DR, contraction 1024 as 4 chunks of 256 (128
    partitions x 2 row-pairs packed in the free dim).
  - scores: per-head contraction 64 as 32 partitions x 2 (q/k stored with
    even/odd head-dims pair-interleaved; head h lives at partition base
    32*(h%4)); the required column permutation of Wq/Wk is folded in on
    the host, 1/8 score scale pre-folded into Wk.
  - exp has a constant -3.8 shift folded in (softmax-invariant) so the
    fp8 et values stay below the e4m3 max of 240 (max logit 8.8).
  - attn @ v: contraction over 512 tokens as 2 chunks of 256 - the exp
    tile's ks-chunk axis doubles as the DoubleRow pair axis, and v is
    stored as [128, 2(chunk), H, 64 v | 64 ones] so the softmax
    denominator rides the same matmul (ones columns), replicated over
    psum rows 64:128 -> reciprocal + multiply, no partition broadcasts.
  - o-projection: ctx written (during the denominator multiply) directly
    into the fp8 pair-interleaved layout the contraction needs; Wo rows
    permuted to match on the host.
Residual x stays f32; LayerNorm rstd = exp(-0.5*ln(var+eps)) so the only
ACT tables used are {exp, ln} - one table set, no reloads. Stats via
bn_stats/bn_aggr. bo folded into x on host; gamma/beta applied on host
(exact: same op order as the reference).

The schedule is software-pipelined: window w+1's q/k/v projection
psum-groups are fed 3 units/head into window w's attention head loop,
window w-1's O-projection pieces run during heads 7-14, and its
LayerNorm finishes at the window boundary. NOTE: emission ORDER is
correctness-critical - a tile read only waits on writes emitted EARLIER
in program order, so every producer unit must be emitted before its
consumer (this is why window 0's v units are part of the prologue).
"""

import numpy as np
import ml_dtypes

import concourse.bacc as bacc
import concourse.mybir as mybir
from concourse.tile import TileContext
from concourse import bass_utils

F32 = mybir.dt.float32
BF16 = mybir.dt.bfloat16
FP8 = mybir.dt.float8e4
ALU = mybir.AluOpType
ACTF = mybir.ActivationFunctionType
DR = mybir.MatmulPerfMode.DoubleRow

E4 = ml_dtypes.float8_e4m3

B, S, D, H, K = 4, 4096, 1024, 16, 64
NW = 8            # windows per sequence
SW = S // NW      # 512
NCORES = 8
NBLK = B * NW     # 32 (batch, window) blocks
WPC = NBLK // NCORES  # 4 blocks per core
CC = 4            # contraction chunks of 256 (d or hk)
SC = SW // 128    # 4 s chunks per window

TRACE = False
LAST_RESULT = None     # BassKernelResults of the last run

_cached_nc = {}


def _build_nc(reps=1):
    # reps > 1 repeats the whole per-window computation (same inputs/outputs)
    # to amplify device time for wall-clock measurement; reps=1 for real runs.
    nc = bacc.Bacc(None, target_bir_lowering=False, debug=False)

    # xT pair layout: [128, CC, 2, SW], d = 256*cc + 2*p + i  (fp8, for v)
    xt_in = nc.dram_tensor("xt", [WPC, 128, CC, 2, SW], FP8, kind="ExternalInput")
    # xT bf16 chunks: [128, 8, SW], d = 128*c + p  (for q/k projections)
    xtb_in = nc.dram_tensor("xtb", [WPC, 128, 2 * CC, SW], BF16, kind="ExternalInput")
    x_in = nc.dram_tensor("x", [WPC, SC, 128, D], F32, kind="ExternalInput")
    # weights pair layouts: [128, CC, 2, 1024] (see host prep for column orders)
    wq_in = nc.dram_tensor("wq", [128, 2 * CC, D], BF16, kind="ExternalInput")
    wk_in = nc.dram_tensor("wk", [128, 2 * CC, D], BF16, kind="ExternalInput")
    wv_in = nc.dram_tensor("wv", [128, CC, 2, D], FP8, kind="ExternalInput")
    wo_in = nc.dram_tensor("wo", [128, CC, 2, D], FP8, kind="ExternalInput")
    out_dram = nc.dram_tensor("out", [WPC, SC, 128, D], F32, kind="ExternalOutput")

    with TileContext(nc) as tc:
        with tc.tile_pool(name="const", bufs=1) as cpool, \
             tc.tile_pool(name="wts", bufs=1) as wpool, \
             tc.tile_pool(name="xt", bufs=2) as xt_pool, \
             tc.tile_pool(name="xnat", bufs=4) as xn_pool, \
             tc.tile_pool(name="qk", bufs=2) as qk_pool, \
             tc.tile_pool(name="et", bufs=4) as e_pool, \
             tc.tile_pool(name="rcp", bufs=2) as r_pool, \
             tc.tile_pool(name="ctx", bufs=2) as c_pool, \
             tc.tile_pool(name="yy", bufs=6) as y_pool, \
             tc.tile_pool(name="oo", bufs=4) as o_pool, \
             tc.tile_pool(name="st", bufs=2) as s_pool, \
             tc.tile_pool(name="ps_proj", bufs=2, space="PSUM") as ps_proj, \
             tc.tile_pool(name="ps_sc", bufs=2, space="PSUM") as ps_sc, \
             tc.tile_pool(name="ps_acc", bufs=2, space="PSUM") as ps_acc:

            # ---- persistent constants ----
            # (weight DMAs are emitted after the first xt load; see below)
            wq_sb = wpool.tile([128, 2 * CC, D], BF16, tag="wq")
            wk_sb = wpool.tile([128, 2 * CC, D], BF16, tag="wk")
            wv_sb = wpool.tile([128, CC, 2, D], FP8, tag="wv")
            wo_sb = wpool.tile([128, CC, 2, D], FP8, tag="wo")
            eps_sb = cpool.tile([128, 1], F32, tag="eps")
            nc.vector.memset(eps_sb, 1e-3)
            # Persistent v tiles (2 chunk-pairs x window parity): the ones
            # columns (denominator trick) are written once via a DVE copy
            # from a memset staging tile (memset->copy->matmul gives the
            # matmul a standard tensor-op dependency edge) and never
            # rewritten; the v halves are overwritten in place each window.
            ones_src = cpool.tile([128, 2, H, 64], FP8, tag="ones_src")
            nc.vector.memset(ones_src, 1.0)
            v_sets = []
            for par in range(2):
                vts = []
                for k2 in range(2):
                    vt = cpool.tile([128, 2, H, 128], FP8, tag=f"v{par}{k2}")
                    nc.vector.tensor_copy(vt[:, :, :, 64:128], ones_src)
                    vts.append(vt)
                v_sets.append(vts)

            seq = [wi for _ in range(reps) for wi in range(WPC)]
            states = {}   # wseq -> projection tiles for that window
            lnst = {}     # wseq -> residual/LN state

            def emit_xt(ws):
                st = {}
                st["xTb"] = xt_pool.tile([128, 2 * CC, SW], BF16, tag="xTb", name="xTb")
                nc.scalar.dma_start(st["xTb"], xtb_in[seq[ws]])
                st["xT"] = xt_pool.tile([128, CC, 2, SW], FP8, tag="xT", name="xT")
                nc.sync.dma_start(st["xT"], xt_in[seq[ws]])
                st["qT"] = qk_pool.tile([128, 4, 2, SW], FP8, tag="qT", name="qT")
                st["kT"] = qk_pool.tile([128, 4, 2, SW], FP8, tag="kT", name="kT")
                states[ws] = st

            def emit_qkv_unit(ws, unit):
                # units 0..31: q/k half-groups (4 bf16 chunk-matmuls);
                # even unit = first half (allocates psum), odd = second+copy.
                # units 32..39: whole v groups (4 fp8-DR matmuls + copy).
                st = states[ws]
                if unit < 32:
                    g, part = unit >> 1, unit & 1
                    j, is_k = g >> 1, g & 1
                    w_sb = wk_sb if is_k else wq_sb
                    dst = st["kT"] if is_k else st["qT"]
                    if part == 0:
                        st["pj"] = ps_proj.tile([128, 512], F32, tag="pp",
                                                name="pj")
                    pj = st["pj"]
                    for c in range(4 * part, 4 * part + 4):
                        nc.tensor.matmul(pj, lhsT=w_sb[:, c, j * 128:(j + 1) * 128],
                                         rhs=st["xTb"][:, c, :],
                                         start=(c == 0), stop=(c == 2 * CC - 1))
                    if part == 1:
                        if is_k and j >= 5:
                            nc.vector.tensor_copy(dst[:, j >> 1, j & 1, :], pj)
                        else:
                            nc.scalar.activation(dst[:, j >> 1, j & 1, :], pj,
                                                 ACTF.Identity)
                else:
                    gv = unit - 32
                    m, half = gv >> 1, gv & 1
                    vt = v_sets[ws % 2][m >> 1]
                    pv = ps_proj.tile([128, 512], F32, tag="pp")
                    for c in range(CC):
                        nc.tensor.matmul(
                            pv, lhsT=st["xT"][:, c, :, m * 128:(m + 1) * 128],
                            rhs=wv_sb[:, c, :, half * 512:(half + 1) * 512],
                            start=(c == 0), stop=(c == CC - 1), perf_mode=DR)
                    nc.vector.tensor_copy(
                        vt[:, m & 1, half * 8:(half + 1) * 8, 0:64],
                        pv.rearrange("p (hh k) -> p hh k", k=64))

            # unit order: q0,k0,q1,k1 halves; v0..v7; remaining q/k halves
            UORDER = ([2 * g + p for g in (0, 1, 2, 3) for p in (0, 1)]
                      + list(range(32, 40))
                      + [2 * g + p for g in range(4, 16) for p in (0, 1)])
            assert len(UORDER) == 40

            def emit_scores_exp(ws, h):
                st = states[ws]
                a32, h4 = 32 * (h % 4), h // 4
                cps = ps_acc.tile([128, 512], F32, tag="acc")
                ets = []
                for k2 in range(2):
                    sps = ps_sc.tile([128, 2, 512], F32, tag="sps")
                    for u in range(2):
                        ks = 2 * k2 + u
                        nc.tensor.matmul(
                            sps[:, u, :],
                            lhsT=st["kT"][a32:a32 + 32, h4, :, ks * 128:(ks + 1) * 128],
                            rhs=st["qT"][a32:a32 + 32, h4, :, :],
                            start=True, stop=True, perf_mode=DR,
                            tile_position=(a32, 0))
                    et = e_pool.tile([128, 2, 512], FP8, tag="exp")
                    nc.scalar.activation(et, sps, ACTF.Exp, bias=c3_sb[:, 0:1])
                    ets.append(et)
                return cps, ets

            def emit_ctx(ws, h, cps, ets, feed):
                st = states[ws]
                cc, a2, i2 = h // 4, (h % 4) // 2, h % 2
                for k2 in range(2):
                    nc.tensor.matmul(cps, lhsT=v_sets[ws % 2][k2][:, :, h, :],
                                     rhs=ets[k2][:, :, :],
                                     start=(k2 == 0), stop=(k2 == 1),
                                     perf_mode=DR)
                    feed()
                rb = r_pool.tile([64, 512], F32, tag="rcp")
                nc.vector.reciprocal(rb, cps[64:128, :])
                nc.vector.tensor_tensor(
                    st["ctx"][64 * a2:64 * a2 + 64, cc, i2, :],
                    cps[0:64, :], rb, op=ALU.mult)
                feed()

            def emit_head(ws, h, feed):
                cps, ets = emit_scores_exp(ws, h)
                emit_ctx(ws, h, cps, ets, feed)

            def emit_x_load(ws, m):
                li = lnst.setdefault(ws, {"x": {}, "y": {}, "bst": {}})
                x_t = xn_pool.tile([128, D], F32, tag="xn")
                nc.sync.dma_start(x_t, x_in[seq[ws], m])
                li["x"][m] = x_t

            def emit_pout_piece(ws, piece):
                m, half = piece >> 1, piece & 1
                li = lnst[ws]
                ctx_t = states[ws]["ctx"]
                if half == 0:
                    li["y"][m] = y_pool.tile([128, D], F32, tag="y", name="y")
                    li["bst"][m] = s_pool.tile([128, 2, 6], F32, tag="bst", name="bst")
                    if m == 0:
                        li["mv4"] = s_pool.tile([128, SC, 2], F32, tag="mv4", name="mv4")
                y_t, bst = li["y"][m], li["bst"][m]
                pout = ps_acc.tile([128, 512], F32, tag="acc")
                for cc2 in range(CC):
                    nc.tensor.matmul(
                        pout, lhsT=ctx_t[:, cc2, :, m * 128:(m + 1) * 128],
                        rhs=wo_sb[:, cc2, :, half * 512:(half + 1) * 512],
                        start=(cc2 == 0), stop=(cc2 == CC - 1), perf_mode=DR)
                nc.vector.tensor_tensor(
                    y_t[:, half * 512:(half + 1) * 512], pout,
                    li["x"][m][:, half * 512:(half + 1) * 512], op=ALU.add)
                nc.vector.bn_stats(bst[:, half, :],
                                   y_t[:, half * 512:(half + 1) * 512])
                if half == 1:
                    nc.vector.bn_aggr(li["mv4"][:, m, :], bst)

            def emit_ln_stats(ws):
                li = lnst[ws]
                mv4 = li["mv4"]
                lnv = s_pool.tile([128, SC], F32, tag="lnv")
                nc.scalar.activation(lnv, mv4[:, :, 1], ACTF.Ln,
                                     bias=eps_sb[:, 0:1])
                rstd4 = s_pool.tile([128, SC], F32, tag="rstd4")
                nc.scalar.activation(rstd4, lnv, ACTF.Exp, scale=-0.5)
                nmr4 = s_pool.tile([128, SC], F32, tag="nmr4")
                nc.vector.scalar_tensor_tensor(nmr4, mv4[:, :, 0], -1.0,
                                               rstd4, ALU.mult, ALU.mult)
                li["rstd4"], li["nmr4"] = rstd4, nmr4

            def emit_o(ws, m, act_ok=True, qalt=False):
                li = lnst[ws]
                o_t = o_pool.tile([128, D], F32, tag="o")
                if act_ok and m % 2 == 0:
                    nc.scalar.activation(o_t, li["y"][m], ACTF.Identity,
                                         bias=li["nmr4"][:, m:m + 1],
                                         scale=li["rstd4"][:, m:m + 1])
                else:
                    nc.vector.tensor_scalar(o_t, li["y"][m],
                                            li["rstd4"][:, m:m + 1],
                                            li["nmr4"][:, m:m + 1],
                                            ALU.mult, ALU.add)
                eng = nc.scalar if qalt and m % 2 else nc.sync
                eng.dma_start(out_dram[seq[ws], m], o_t)
                if m == SC - 1:
                    del lnst[ws]

            # ---- software-pipelined schedule ----
            n = len(seq)
            emit_xt(0)
            # q/k weights chunked, ordered by first use: half1 cols (heads
            # 0-7's psums), then xt/wv (v path), then half2, then wo
            for c in range(2 * CC):
                nc.scalar.dma_start(wq_sb[:, c, 0:512], wq_in[:, c, 0:512])
                nc.sync.dma_start(wk_sb[:, c, 0:512], wk_in[:, c, 0:512])
            nc.gpsimd.dma_start(wv_sb, wv_in[:, :, :, :])
            for c in range(2 * CC):
                nc.scalar.dma_start(wq_sb[:, c, 512:D], wq_in[:, c, 512:D])
                nc.sync.dma_start(wk_sb[:, c, 512:D], wk_in[:, c, 512:D])
            nc.scalar.dma_start(wo_sb, wo_in[:, :, :, :])
            states[0]["ctx"] = c_pool.tile([128, CC, 2, SW], FP8, tag="ctx",
                                           name="ctx")
            for u in UORDER[:16]:
                emit_qkv_unit(0, u)
            feed_q = list(UORDER[16:])
            feed_ws = [0] * len(feed_q)

            for i in range(n):
                if i > 0:
                    states[i]["ctx"] = c_pool.tile([128, CC, 2, SW], FP8,
                                                   tag="ctx", name="ctx")

                def feed():
                    if feed_q:
                        emit_qkv_unit(feed_ws.pop(0), feed_q.pop(0))

                for h in range(H):
                    emit_head(i, h, feed)
                    if i + 1 < n and h == 0:
                        emit_xt(i + 1)
                        feed_q.extend(UORDER)
                        feed_ws.extend([i + 1] * len(UORDER))
                    if i > 0:
                        if 3 <= h < 7:
                            emit_x_load(i - 1, h - 3)
                        if 7 <= h < 15:
                            emit_pout_piece(i - 1, h - 7)
                    if i == n - 1 and 11 <= h < 15:
                        emit_x_load(i, h - 11)
                if i > 0:
                    emit_ln_stats(i - 1)
                    for m in range(SC):
                        emit_o(i - 1, m)
            for piece in range(8):
                emit_pout_piece(n - 1, piece)
            emit_ln_stats(n - 1)
            for m in range(SC):
                emit_o(n - 1, m, act_ok=False, qalt=True)

    nc.compile()
    return nc


def _get_nc(reps=1):
    if reps not in _cached_nc:
        _cached_nc[reps] = _build_nc(reps=reps)
    return _cached_nc[reps]


def _prep_inputs(x, Wq, bq, Wk, bk, Wv, bv, Wo, bo):
    """Host-side quantization, layouts, and column/row permutations."""
    f32 = np.float32
    xb = x.reshape(NBLK, SW, D)
    if np.any(bo):
        xb = xb + bo
    x_nat = np.ascontiguousarray(xb.reshape(NBLK, SC, 128, D), f32)

    # xT pair layout [NBLK, 128, CC, 2, SW]: d = 256*cc + 2*p + i
    xTd = xb.transpose(0, 2, 1)
    xT = xTd.reshape(NBLK, CC, 128, 2, SW)
    xT = np.ascontiguousarray(xT.transpose(0, 2, 1, 3, 4)).astype(E4)
    # bf16 d-major chunks [NBLK, 128, 8, SW]: d = 128*c + p
    xTb = xTd.reshape(NBLK, 2 * CC, 128, SW)
    xTb = np.ascontiguousarray(xTb.transpose(0, 2, 1, 3)).astype(ml_dtypes.bfloat16)

    # q/k column permutation: psum j=(h4,kp), col p' -> q-dim
    # (head 4*h4 + p'//32, k = 2*(p'%32) + kp)
    j_idx = np.arange(8)
    h4, kp = j_idx >> 1, j_idx & 1
    pp = np.arange(128)
    col_src = ((4 * h4[:, None] + pp[None, :] // 32) * 64
               + 2 * (pp[None, :] % 32) + kp[:, None])  # [8, 128]
    perm = col_src.reshape(-1)

    def pair_rows(wmat):
        # [D, 1024cols] -> [128, CC, 2, 1024]: row d = 256*cc + 2*p + i
        wr = wmat.reshape(CC, 128, 2, D)
        return np.ascontiguousarray(wr.transpose(1, 0, 2, 3))

    def chunk_rows(wmat):
        # [D, 1024cols] -> [128, 8, 1024]: row d = 128*c + p
        wr = wmat.reshape(2 * CC, 128, D)
        return np.ascontiguousarray(wr.transpose(1, 0, 2))

    wq2 = Wq.reshape(D, H * K)[:, perm]
    wk2 = (Wk.reshape(D, H * K) * 0.125)[:, perm]
    wv2 = Wv.reshape(D, H * K)
    wq_p = chunk_rows(wq2).astype(ml_dtypes.bfloat16)
    wk_p = chunk_rows(wk2).astype(ml_dtypes.bfloat16)
    wv_p = pair_rows(wv2).astype(E4)

    # Wo rows: [128, CC, 2, D]: row (p=64*a+vk, cc, i) = Wo[(4cc+2a+i)*64+vk]
    wo_r = Wo.reshape(CC, 2, 2, 64, D)          # [cc, a, i, vk, D]
    wo_p = np.ascontiguousarray(
        wo_r.transpose(1, 3, 0, 2, 4).reshape(128, CC, 2, D)).astype(E4)
    return x_nat, xT, xTb, wq_p, wk_p, wv_p, wo_p


def kernel(x, Wq, bq, Wk, bk, Wv, bv, Wo, bo, gamma, beta, num_window):
    global LAST_RESULT
    x = np.ascontiguousarray(np.asarray(x, dtype=np.float32))
    Wq = np.asarray(Wq, np.float32)
    Wk = np.asarray(Wk, np.float32)
    Wv = np.asarray(Wv, np.float32)
    Wo = np.asarray(Wo, np.float32)
    bq = np.asarray(bq, np.float32).reshape(H * K)
    bk = np.asarray(bk, np.float32).reshape(H * K)
    bv = np.asarray(bv, np.float32).reshape(H * K)
    bo = np.asarray(bo, np.float32).reshape(D)
    gamma = np.asarray(gamma, np.float32).reshape(D)
    beta = np.asarray(beta, np.float32).reshape(D)
    assert int(num_window) == NW, f"kernel compiled for num_window={NW}"
    assert x.shape == (B, S, D)
    assert not (np.any(bq) or np.any(bk) or np.any(bv)), \
        "fp8 fast path assumes zero qkv biases"

    x_nat, xT, xTb, wq_p, wk_p, wv_p, wo_p = _prep_inputs(
        x, Wq, bq, Wk, bk, Wv, bv, Wo, bo)

    shared = {"wq": wq_p, "wk": wk_p, "wv": wv_p, "wo": wo_p}
    in_maps = []
    for c in range(NCORES):
        m = dict(shared)
        m["xt"] = np.ascontiguousarray(xT[c * WPC:(c + 1) * WPC])
        m["xtb"] = np.ascontiguousarray(xTb[c * WPC:(c + 1) * WPC])
        m["x"] = np.ascontiguousarray(x_nat[c * WPC:(c + 1) * WPC])
        in_maps.append(m)

    nc = _get_nc()
    res = bass_utils.run_bass_kernel_spmd(
        nc, in_maps, core_ids=list(range(NCORES)), trace=TRACE)
    LAST_RESULT = res

    y = np.empty((NBLK, SC, 128, D), np.float32)
    for c in range(NCORES):
        y[c * WPC:(c + 1) * WPC] = res.results[c]["out"]
    y = y.reshape(B, S, D)
    if np.any(gamma != 1.0) or np.any(beta):
        y = y * gamma + beta
    return y


# revision 29
# speedup vs baseline: 1.0018x; 1.0018x over previous
"""Localized (block-diagonal windowed) self-attention + residual + LayerNorm
on 8 Trainium2 NeuronCores.

Problem (hardcoded): x [B=4, S=4096, D=1024], H=16 heads, K=64 head dim,
num_window=8 -> window length Sw=512. Per (batch, window) block:
    q/k/v = xw @ W* + b*          [512, 16, 64]
    scores = q k^T / 8 per head   [512, 512]
    attn = softmax(scores)
    ctx = attn @ v
    attn_out = ctx @ Wo + bo
    out = LayerNorm(x + attn_out) * gamma + beta   (eps=1e-3)

Sharding: pure data parallelism over the 32 (batch, window) blocks, 4 per
core; weights replicated. No collectives.

Device strategy (v2): fp8-e4m3 DoubleRow perf mode (0.5 cycles/row)
everywhere the ~2e-2 error budget allows; bf16 where it does not:
  - q/k projections: bf16 (fp8 Wq/Wk quantization perturbs softmax
    weights enough to fail the tolerance on peaked-attention tokens);
    q/k outputs stored fp8 in the scores pair layout.
  - v/o projections: fp8 DR, contraction 1024 as 4 chunks of 256 (128
    partitions x 2 row-pairs packed in the free dim).
  - scores: per-head contraction 64 as 32 partitions x 2 (q/k stored with
    even/odd head-dims pair-interleaved; head h lives at partition base
    32*(h%4)); the required column permutation of Wq/Wk is folded in on
    the host, 1/8 score scale pre-folded into Wk.
  - exp has a constant -3.8 shift folded in (softmax-invariant) so the
    fp8 et values stay below the e4m3 max of 240 (max logit 8.8).
  - attn @ v: contraction over 512 tokens as 2 chunks of 256 - the exp
    tile's ks-chunk axis doubles as the DoubleRow pair axis, and v is
    stored as [128, 2(chunk), H, 64 v | 64 ones] so the softmax
    denominator rides the same matmul (ones columns), replicated over
    psum rows 64:128 -> reciprocal + multiply, no partition broadcasts.
  - o-projection: ctx written (during the denominator multiply) directly
    into the fp8 pair-interleaved layout the contraction needs; Wo rows
    permuted to match on the host.
Residual x stays f32; LayerNorm rstd = exp(-0.5*ln(var+eps)) so the only
ACT tables used are {exp, ln} - one table set, no reloads. Stats via
bn_stats/bn_aggr. bo folded into x on host; gamma/beta applied on host
(exact: same op order as the reference).

The schedule is software-pipelined: window w+1's q/k/v projection
psum-groups are fed 3 units/head into window w's attention head loop,
window w-1's O-projection pieces run during heads 7-14, and its
LayerNorm finishes at the window boundary. NOTE: emission ORDER is
correctness-critical - a tile read only waits on writes emitted EARLIER
in program order, so every producer unit must be emitted before its
consumer (this is why window 0's v units are part of the prologue).
"""

import numpy as np
import ml_dtypes

import concourse.bacc as bacc
import concourse.mybir as mybir
from concourse.tile import TileContext
from concourse import bass_utils

F32 = mybir.dt.float32
BF16 = mybir.dt.bfloat16
FP8 = mybir.dt.float8e4
ALU = mybir.AluOpType
ACTF = mybir.ActivationFunctionType
DR = mybir.MatmulPerfMode.DoubleRow

E4 = ml_dtypes.float8_e4m3

B, S, D, H, K = 4, 4096, 1024, 16, 64
NW = 8            # windows per sequence
SW = S // NW      # 512
NCORES = 8
NBLK = B * NW     # 32 (batch, window) blocks
WPC = NBLK // NCORES  # 4 blocks per core
CC = 4            # contraction chunks of 256 (d or hk)
SC = SW // 128    # 4 s chunks per window

TRACE = False
LAST_RESULT = None     # BassKernelResults of the last run

_cached_nc = {}


def _build_nc(reps=1):
    # reps > 1 repeats the whole per-window computation (same inputs/outputs)
    # to amplify device time for wall-clock measurement; reps=1 for real runs.
    nc = bacc.Bacc(None, target_bir_lowering=False, debug=False)

    # xT pair layout: [128, CC, 2, SW], d = 256*cc + 2*p + i  (fp8, for v)
    xt_in = nc.dram_tensor("xt", [WPC, 128, CC, 2, SW], FP8, kind="ExternalInput")
    # xT bf16 chunks: [128, 8, SW], d = 128*c + p  (for q/k projections)
    xtb_in = nc.dram_tensor("xtb", [WPC, 128, 2 * CC, SW], BF16, kind="ExternalInput")
    x_in = nc.dram_tensor("x", [WPC, SC, 128, D], F32, kind="ExternalInput")
    # weights pair layouts: [128, CC, 2, 1024] (see host prep for column orders)
    wq_in = nc.dram_tensor("wq", [128, 2 * CC, D], BF16, kind="ExternalInput")
    wk_in = nc.dram_tensor("wk", [128, 2 * CC, D], BF16, kind="ExternalInput")
    wv_in = nc.dram_tensor("wv", [128, CC, 2, D], FP8, kind="ExternalInput")
    wo_in = nc.dram_tensor("wo", [128, CC, 2, D], FP8, kind="ExternalInput")
    out_dram = nc.dram_tensor("out", [WPC, SC, 128, D], F32, kind="ExternalOutput")

    with TileContext(nc) as tc:
        with tc.tile_pool(name="const", bufs=1) as cpool, \
             tc.tile_pool(name="wts", bufs=1) as wpool, \
             tc.tile_pool(name="xt", bufs=2) as xt_pool, \
             tc.tile_pool(name="xnat", bufs=5) as xn_pool, \
             tc.tile_pool(name="qk", bufs=2) as qk_pool, \
             tc.tile_pool(name="et", bufs=6) as e_pool, \
             tc.tile_pool(name="rcp", bufs=3) as r_pool, \
             tc.tile_pool(name="ctx", bufs=2) as c_pool, \
             tc.tile_pool(name="yy", bufs=7) as y_pool, \
             tc.tile_pool(name="oo", bufs=5) as o_pool, \
             tc.tile_pool(name="st", bufs=3) as s_pool, \
             tc.tile_pool(name="ps_proj", bufs=2, space="PSUM") as ps_proj, \
             tc.tile_pool(name="ps_sc", bufs=2, space="PSUM") as ps_sc, \
             tc.tile_pool(name="ps_acc", bufs=2, space="PSUM") as ps_acc:

            # ---- persistent constants ----
            # (weight DMAs are emitted after the first xt load; see below)
            wq_sb = wpool.tile([128, 2 * CC, D], BF16, tag="wq")
            wk_sb = wpool.tile([128, 2 * CC, D], BF16, tag="wk")
            wv_sb = wpool.tile([128, CC, 2, D], FP8, tag="wv")
            wo_sb = wpool.tile([128, CC, 2, D], FP8, tag="wo")
            eps_sb = cpool.tile([128, 1], F32, tag="eps")
            nc.vector.memset(eps_sb, 1e-3)
            # Persistent v tiles (2 chunk-pairs x window parity): the ones
            # columns (denominator trick) are written once via a DVE copy
            # from a memset staging tile (memset->copy->matmul gives the
            # matmul a standard tensor-op dependency edge) and never
            # rewritten; the v halves are overwritten in place each window.
            ones_src = cpool.tile([128, 2, H, 64], FP8, tag="ones_src")
            nc.vector.memset(ones_src, 1.0)
            v_sets = []
            for par in range(2):
                vts = []
                for k2 in range(2):
                    vt = cpool.tile([128, 2, H, 128], FP8, tag=f"v{par}{k2}")
                    nc.vector.tensor_copy(vt[:, :, :, 64:128], ones_src)
                    vts.append(vt)
                v_sets.append(vts)

            seq = [wi for _ in range(reps) for wi in range(WPC)]
            states = {}   # wseq -> projection tiles for that window
            lnst = {}     # wseq -> residual/LN state

            def emit_xt(ws):
                st = {}
                st["xTb"] = xt_pool.tile([128, 2 * CC, SW], BF16, tag="xTb", name="xTb")
                nc.scalar.dma_start(st["xTb"], xtb_in[seq[ws]])
                st["xT"] = xt_pool.tile([128, CC, 2, SW], FP8, tag="xT", name="xT")
                nc.sync.dma_start(st["xT"], xt_in[seq[ws]])
                st["qT"] = qk_pool.tile([128, 4, 2, SW], FP8, tag="qT", name="qT")
                st["kT"] = qk_pool.tile([128, 4, 2, SW], FP8, tag="kT", name="kT")
                states[ws] = st

            def emit_qkv_unit(ws, unit):
                # units 0..31: q/k half-groups (4 bf16 chunk-matmuls);
                # even unit = first half (allocates psum), odd = second+copy.
                # units 32..39: whole v groups (4 fp8-DR matmuls + copy).
                st = states[ws]
                if unit < 32:
                    g, part = unit >> 1, unit & 1
                    j, is_k = g >> 1, g & 1
                    w_sb = wk_sb if is_k else wq_sb
                    dst = st["kT"] if is_k else st["qT"]
                    if part == 0:
                        st["pj"] = ps_proj.tile([128, 512], F32, tag="pp",
                                                name="pj")
                    pj = st["pj"]
                    for c in range(4 * part, 4 * part + 4):
                        nc.tensor.matmul(pj, lhsT=w_sb[:, c, j * 128:(j + 1) * 128],
                                         rhs=st["xTb"][:, c, :],
                                         start=(c == 0), stop=(c == 2 * CC - 1))
                    if part == 1:
                        if is_k and j >= 5:
                            nc.vector.tensor_copy(dst[:, j >> 1, j & 1, :], pj)
                        else:
                            nc.scalar.activation(dst[:, j >> 1, j & 1, :], pj,
                                                 ACTF.Identity)
                else:
                    gv = unit - 32
                    m, half = gv >> 1, gv & 1
                    vt = v_sets[ws % 2][m >> 1]
                    pv = ps_proj.tile([128, 512], F32, tag="pp")
                    for c in range(CC):
                        nc.tensor.matmul(
                            pv, lhsT=st["xT"][:, c, :, m * 128:(m + 1) * 128],
                            rhs=wv_sb[:, c, :, half * 512:(half + 1) * 512],
                            start=(c == 0), stop=(c == CC - 1), perf_mode=DR)
                    nc.vector.tensor_copy(
                        vt[:, m & 1, half * 8:(half + 1) * 8, 0:64],
                        pv.rearrange("p (hh k) -> p hh k", k=64))

            # unit order: q0,k0,q1,k1 halves; v0..v7; remaining q/k halves
            UORDER = ([2 * g + p for g in (0, 1, 2, 3) for p in (0, 1)]
                      + list(range(32, 40))
                      + [2 * g + p for g in range(4, 16) for p in (0, 1)])
            assert len(UORDER) == 40

            def emit_scores_exp(ws, h):
                st = states[ws]
                a32, h4 = 32 * (h % 4), h // 4
                cps = ps_acc.tile([128, 512], F32, tag="acc")
                ets = []
                for k2 in range(2):
                    sps = ps_sc.tile([128, 2, 512], F32, tag="sps")
                    for u in range(2):
                        ks = 2 * k2 + u
                        nc.tensor.matmul(
                            sps[:, u, :],
                            lhsT=st["kT"][a32:a32 + 32, h4, :, ks * 128:(ks + 1) * 128],
                            rhs=st["qT"][a32:a32 + 32, h4, :, :],
                            start=True, stop=True, perf_mode=DR,
                            tile_position=(a32, 0))
                    et = e_pool.tile([128, 2, 512], FP8, tag="exp")
                    nc.scalar.activation(et, sps, ACTF.Exp, bias=c3_sb[:, 0:1])
                    ets.append(et)
                return cps, ets

            def emit_ctx(ws, h, cps, ets, feed):
                st = states[ws]
                cc, a2, i2 = h // 4, (h % 4) // 2, h % 2
                for k2 in range(2):
                    nc.tensor.matmul(cps, lhsT=v_sets[ws % 2][k2][:, :, h, :],
                                     rhs=ets[k2][:, :, :],
                                     start=(k2 == 0), stop=(k2 == 1),
                                     perf_mode=DR)
                    feed()
                rb = r_pool.tile([64, 512], F32, tag="rcp")
                nc.vector.reciprocal(rb, cps[64:128, :])
                nc.vector.tensor_tensor(
                    st["ctx"][64 * a2:64 * a2 + 64, cc, i2, :],
                    cps[0:64, :], rb, op=ALU.mult)
                feed()

            def emit_head(ws, h, feed):
                cps, ets = emit_scores_exp(ws, h)
                emit_ctx(ws, h, cps, ets, feed)

            def emit_x_load(ws, m):
                li = lnst.setdefault(ws, {"x": {}, "y": {}, "bst": {}})
                x_t = xn_pool.tile([128, D], F32, tag="xn")
                nc.sync.dma_start(x_t, x_in[seq[ws], m])
                li["x"][m] = x_t

            def emit_pout_piece(ws, piece):
                m, half = piece >> 1, piece & 1
                li = lnst[ws]
                ctx_t = states[ws]["ctx"]
                if half == 0:
                    li["y"][m] = y_pool.tile([128, D], F32, tag="y", name="y")
                    li["bst"][m] = s_pool.tile([128, 2, 6], F32, tag="bst", name="bst")
                    if m == 0:
                        li["mv4"] = s_pool.tile([128, SC, 2], F32, tag="mv4", name="mv4")
                y_t, bst = li["y"][m], li["bst"][m]
                pout = ps_acc.tile([128, 512], F32, tag="acc")
                for cc2 in range(CC):
                    nc.tensor.matmul(
                        pout, lhsT=ctx_t[:, cc2, :, m * 128:(m + 1) * 128],
                        rhs=wo_sb[:, cc2, :, half * 512:(half + 1) * 512],
                        start=(cc2 == 0), stop=(cc2 == CC - 1), perf_mode=DR)
                nc.vector.tensor_tensor(
                    y_t[:, half * 512:(half + 1) * 512], pout,
                    li["x"][m][:, half * 512:(half + 1) * 512], op=ALU.add)
                nc.vector.bn_stats(bst[:, half, :],
                                   y_t[:, half * 512:(half + 1) * 512])
                if half == 1:
                    nc.vector.bn_aggr(li["mv4"][:, m, :], bst)

            def emit_ln_stats(ws):
                li = lnst[ws]
                mv4 = li["mv4"]
                lnv = s_pool.tile([128, SC], F32, tag="lnv")
                nc.scalar.activation(lnv, mv4[:, :, 1], ACTF.Ln,
                                     bias=eps_sb[:, 0:1])
                rstd4 = s_pool.tile([128, SC], F32, tag="rstd4")
                nc.scalar.activation(rstd4, lnv, ACTF.Exp, scale=-0.5)
                nmr4 = s_pool.tile([128, SC], F32, tag="nmr4")
                nc.vector.scalar_tensor_tensor(nmr4, mv4[:, :, 0], -1.0,
                                               rstd4, ALU.mult, ALU.mult)
                li["rstd4"], li["nmr4"] = rstd4, nmr4

            def emit_o(ws, m, act_ok=True, qalt=False):
                li = lnst[ws]
                o_t = o_pool.tile([128, D], F32, tag="o")
                if act_ok and m % 2 == 0:
                    nc.scalar.activation(o_t, li["y"][m], ACTF.Identity,
                                         bias=li["nmr4"][:, m:m + 1],
                                         scale=li["rstd4"][:, m:m + 1])
                else:
                    nc.vector.tensor_scalar(o_t, li["y"][m],
                                            li["rstd4"][:, m:m + 1],
                                            li["nmr4"][:, m:m + 1],
                                            ALU.mult, ALU.add)
                eng = nc.scalar if qalt and m % 2 else nc.sync
                eng.dma_start(out_dram[seq[ws], m], o_t)
                if m == SC - 1:
                    del lnst[ws]

            # ---- software-pipelined schedule ----
            n = len(seq)
            emit_xt(0)
            # q/k weights chunked, ordered by first use: half1 cols (heads
            # 0-7's psums), then xt/wv (v path), then half2, then wo
            for c in range(2 * CC):
                nc.scalar.dma_start(wq_sb[:, c, 0:512], wq_in[:, c, 0:512])
                nc.sync.dma_start(wk_sb[:, c, 0:512], wk_in[:, c, 0:512])
            nc.gpsimd.dma_start(wv_sb, wv_in[:, :, :, :])
            for c in range(2 * CC):
                nc.scalar.dma_start(wq_sb[:, c, 512:D], wq_in[:, c, 512:D])
                nc.sync.dma_start(wk_sb[:, c, 512:D], wk_in[:, c, 512:D])
            nc.scalar.dma_start(wo_sb, wo_in[:, :, :, :])
            states[0]["ctx"] = c_pool.tile([128, CC, 2, SW], FP8, tag="ctx",
                                           name="ctx")
            for u in UORDER[:16]:
                emit_qkv_unit(0, u)
            feed_q = list(UORDER[16:])
            feed_ws = [0] * len(feed_q)

            for i in range(n):
                if i > 0:
                    states[i]["ctx"] = c_pool.tile([128, CC, 2, SW], FP8,
                                                   tag="ctx", name="ctx")

                def feed():
                    if feed_q:
                        emit_qkv_unit(feed_ws.pop(0), feed_q.pop(0))

                for h in range(H):
                    emit_head(i, h, feed)
                    if i + 1 < n and h == 0:
                        emit_xt(i + 1)
                        feed_q.extend(UORDER)
                        feed_ws.extend([i + 1] * len(UORDER))
                    if i > 0:
                        if 3 <= h < 7:
                            emit_x_load(i - 1, h - 3)
                        if 7 <= h < 15:
                            emit_pout_piece(i - 1, h - 7)
                    if i == n - 1 and 11 <= h < 15:
                        emit_x_load(i, h - 11)
                if i > 0:
                    emit_ln_stats(i - 1)
                    for m in range(SC):
                        emit_o(i - 1, m)
            for piece in range(8):
                emit_pout_piece(n - 1, piece)
            emit_ln_stats(n - 1)
            for m in range(SC):
                emit_o(n - 1, m, act_ok=False, qalt=True)

    nc.compile()
    return nc


def _get_nc(reps=1):
    if reps not in _cached_nc:
        _cached_nc[reps] = _build_nc(reps=reps)
    return _cached_nc[reps]


def _prep_inputs(x, Wq, bq, Wk, bk, Wv, bv, Wo, bo):
    """Host-side quantization, layouts, and column/row permutations."""
    f32 = np.float32
    xb = x.reshape(NBLK, SW, D)
    if np.any(bo):
        xb = xb + bo
    x_nat = np.ascontiguousarray(xb.reshape(NBLK, SC, 128, D), f32)

    # xT pair layout [NBLK, 128, CC, 2, SW]: d = 256*cc + 2*p + i
    xTd = xb.transpose(0, 2, 1)
    xT = xTd.reshape(NBLK, CC, 128, 2, SW)
    xT = np.ascontiguousarray(xT.transpose(0, 2, 1, 3, 4)).astype(E4)
    # bf16 d-major chunks [NBLK, 128, 8, SW]: d = 128*c + p
    xTb = xTd.reshape(NBLK, 2 * CC, 128, SW)
    xTb = np.ascontiguousarray(xTb.transpose(0, 2, 1, 3)).astype(ml_dtypes.bfloat16)

    # q/k column permutation: psum j=(h4,kp), col p' -> q-dim
    # (head 4*h4 + p'//32, k = 2*(p'%32) + kp)
    j_idx = np.arange(8)
    h4, kp = j_idx >> 1, j_idx & 1
    pp = np.arange(128)
    col_src = ((4 * h4[:, None] + pp[None, :] // 32) * 64
               + 2 * (pp[None, :] % 32) + kp[:, None])  # [8, 128]
    perm = col_src.reshape(-1)

    def pair_rows(wmat):
        # [D, 1024cols] -> [128, CC, 2, 1024]: row d = 256*cc + 2*p + i
        wr = wmat.reshape(CC, 128, 2, D)
        return np.ascontiguousarray(wr.transpose(1, 0, 2, 3))

    def chunk_rows(wmat):
        # [D, 1024cols] -> [128, 8, 1024]: row d = 128*c + p
        wr = wmat.reshape(2 * CC, 128, D)
        return np.ascontiguousarray(wr.transpose(1, 0, 2))

    wq2 = Wq.reshape(D, H * K)[:, perm]
    wk2 = (Wk.reshape(D, H * K) * 0.125)[:, perm]
    wv2 = Wv.reshape(D, H * K)
    wq_p = chunk_rows(wq2).astype(ml_dtypes.bfloat16)
    wk_p = chunk_rows(wk2).astype(ml_dtypes.bfloat16)
    wv_p = pair_rows(wv2).astype(E4)

    # Wo rows: [128, CC, 2, D]: row (p=64*a+vk, cc, i) = Wo[(4cc+2a+i)*64+vk]
    wo_r = Wo.reshape(CC, 2, 2, 64, D)          # [cc, a, i, vk, D]
    wo_p = np.ascontiguousarray(
        wo_r.transpose(1, 3, 0, 2, 4).reshape(128, CC, 2, D)).astype(E4)
    return x_nat, xT, xTb, wq_p, wk_p, wv_p, wo_p


def kernel(x, Wq, bq, Wk, bk, Wv, bv, Wo, bo, gamma, beta, num_window):
    global LAST_RESULT
    x = np.ascontiguousarray(np.asarray(x, dtype=np.float32))
    Wq = np.asarray(Wq, np.float32)
    Wk = np.asarray(Wk, np.float32)
    Wv = np.asarray(Wv, np.float32)
    Wo = np.asarray(Wo, np.float32)
    bq = np.asarray(bq, np.float32).reshape(H * K)
    bk = np.asarray(bk, np.float32).reshape(H * K)
    bv = np.asarray(bv, np.float32).reshape(H * K)
    bo = np.asarray(bo, np.float32).reshape(D)
    gamma = np.asarray(gamma, np.float32).reshape(D)
    beta = np.asarray(beta, np.float32).reshape(D)
    assert int(num_window) == NW, f"kernel compiled for num_window={NW}"
    assert x.shape == (B, S, D)
    assert not (np.any(bq) or np.any(bk) or np.any(bv)), \
        "fp8 fast path assumes zero qkv biases"

    x_nat, xT, xTb, wq_p, wk_p, wv_p, wo_p = _prep_inputs(
        x, Wq, bq, Wk, bk, Wv, bv, Wo, bo)

    shared = {"wq": wq_p, "wk": wk_p, "wv": wv_p, "wo": wo_p}
    in_maps = []
    for c in range(NCORES):
        m = dict(shared)
        m["xt"] = np.ascontiguousarray(xT[c * WPC:(c + 1) * WPC])
        m["xtb"] = np.ascontiguousarray(xTb[c * WPC:(c + 1) * WPC])
        m["x"] = np.ascontiguousarray(x_nat[c * WPC:(c + 1) * WPC])
        in_maps.append(m)

    nc = _get_nc()
    res = bass_utils.run_bass_kernel_spmd(
        nc, in_maps, core_ids=list(range(NCORES)), trace=TRACE)
    LAST_RESULT = res

    y = np.empty((NBLK, SC, 128, D), np.float32)
    for c in range(NCORES):
        y[c * WPC:(c + 1) * WPC] = res.results[c]["out"]
    y = y.reshape(B, S, D)
    if np.any(gamma != 1.0) or np.any(beta):
        y = y * gamma + beta
    return y
